# revision 8
# baseline (speedup 1.0000x reference)
"""Trainium2 Bass kernel for nn_Char2Token2Mention (gather + segment-sum).

    ft = token_ft[token_code]               # [NNZ, D] gather
    weighted = ft * spm_vals[:, None]
    out = segment_sum(weighted, spm_rows, num_segments=N_MENTIONS)

Strategy (8-core SPMD, mentions sharded):
  - core i owns mentions [i*8192, (i+1)*8192); spm_rows is sorted so its nnz
    form a contiguous slice.
  - host bins nnz into 128-mention windows (64 per core), pads each window to
    cpw chunks of 128 nnz (pad entries: code 0 with val 0, so they fetch a
    real row that the sel matrix zeroes out -- no bounds check needed).
  - host casts token_ft to bf16 (tolerance is 2e-2; bf16 lands ~2e-3) --
    halves gather DMA traffic and runs the PE at bf16 rate (f32 PSUM).
  - device, per chunk: indirect-DMA gather of 128 rows [128, 256] bf16 from
    the (replicated) token_ft table; DVE builds sel[j, m] = (iota==row_j)
    *val_j; PE matmul sel.T @ ft accumulates the window's [128, 256] f32
    output in PSUM.
  - per window: PSUM -> SBUF copy -> DMA to the core's [8192, 256] output.
  - host concatenates the 8 core outputs.
"""
import os
import numpy as np
import ml_dtypes

import concourse.bacc as bacc
import concourse.bass as bass
import concourse.mybir as mybir
import concourse.tile as tile
from concourse.bass_utils import run_bass_kernel_spmd

P = 128
D = 256
N_TOKENS = 262144
NNZ = 524288
N_MENTIONS = 65536
N_CORES = 8
MENT_PER_CORE = N_MENTIONS // N_CORES          # 8192
WIN_PER_CORE = MENT_PER_CORE // P              # 64
N_WINDOWS = N_MENTIONS // P                    # 512
MIN_CPW = 9

# Results of the last run (set by kernel()); test.py reads exec_time_ns.
LAST_RESULTS = None

_nc_cache = {}


def _build_nc(cpw: int) -> bass.Bass:
    n_chunks = WIN_PER_CORE * cpw
    bf16 = mybir.dt.bfloat16
    nc = bacc.Bacc("TRN2", target_bir_lowering=False, debug=False)
    table = nc.declare_dram_parameter(
        "token_ft", [N_TOKENS, D], bf16, isOutput=False
    )
    codes = nc.declare_dram_parameter(
        "codes", [P, n_chunks], mybir.dt.int32, isOutput=False
    )
    rows = nc.declare_dram_parameter(
        "rows", [P, n_chunks], mybir.dt.float32, isOutput=False
    )
    vals = nc.declare_dram_parameter(
        "vals", [P, n_chunks], mybir.dt.float32, isOutput=False
    )
    iota = nc.declare_dram_parameter(
        "iota", [P, P], mybir.dt.float32, isOutput=False
    )
    out = nc.declare_dram_parameter(
        "out", [MENT_PER_CORE, D], mybir.dt.float32, isOutput=True
    )

    with tile.TileContext(nc) as tc:
        with (
            tc.tile_pool(name="const", bufs=1) as const_pool,
            tc.tile_pool(name="ftp", bufs=32) as ft_pool,
            tc.tile_pool(name="selp", bufs=16) as sel_pool,
            tc.tile_pool(name="psum", bufs=4, space="PSUM") as psum_pool,
            tc.tile_pool(name="outp", bufs=4) as out_pool,
        ):
            codes_sb = const_pool.tile([P, n_chunks], mybir.dt.int32)
            rows_sb = const_pool.tile([P, n_chunks], mybir.dt.float32)
            vals_sb = const_pool.tile([P, n_chunks], mybir.dt.float32)
            iota_sb = const_pool.tile([P, P], mybir.dt.float32)
            nc.sync.dma_start(out=codes_sb[:], in_=codes[:])
            nc.sync.dma_start(out=rows_sb[:], in_=rows[:])
            nc.sync.dma_start(out=vals_sb[:], in_=vals[:])
            nc.sync.dma_start(out=iota_sb[:], in_=iota[:])

            for w in range(WIN_PER_CORE):
                psum = psum_pool.tile(
                    [P, D], mybir.dt.float32, space="PSUM", tag="acc"
                )
                for c in range(cpw):
                    k = w * cpw + c
                    ft = ft_pool.tile([P, D], bf16, tag="ft")
                    nc.gpsimd.indirect_dma_start(
                        out=ft[:],
                        out_offset=None,
                        in_=table[:],
                        in_offset=bass.IndirectOffsetOnAxis(
                            ap=codes_sb[:, k : k + 1], axis=0
                        ),
                    )
                    sel = sel_pool.tile([P, P], bf16, tag="sel")
                    nc.vector.tensor_scalar(
                        out=sel[:],
                        in0=iota_sb[:],
                        scalar1=rows_sb[:, k : k + 1],
                        scalar2=vals_sb[:, k : k + 1],
                        op0=mybir.AluOpType.is_equal,
                        op1=mybir.AluOpType.mult,
                    )
                    nc.tensor.matmul(
                        out=psum[:],
                        lhsT=sel[:],
                        rhs=ft[:],
                        start=(c == 0),
                        stop=(c == cpw - 1),
                    )
                outt = out_pool.tile([P, D], mybir.dt.float32, tag="out")
                nc.vector.tensor_copy(out=outt[:], in_=psum[:])
                nc.sync.dma_start(out=out[w * P : (w + 1) * P, :], in_=outt[:])
    nc.compile()
    return nc


def kernel(token_ft, token_code, spm_rows, spm_vals):
    global LAST_RESULTS
    ft = np.ascontiguousarray(
        np.asarray(token_ft, dtype=np.float32).astype(ml_dtypes.bfloat16)
    )
    codes = np.asarray(token_code).astype(np.int64, copy=False)
    rows = np.asarray(spm_rows).astype(np.int64, copy=False)
    vals = np.asarray(spm_vals, dtype=np.float32)
    if not np.all(rows[:-1] <= rows[1:]):
        order = np.argsort(rows, kind="stable")
        rows, codes, vals = rows[order], codes[order], vals[order]

    # bin nnz into 128-mention windows (rows is sorted)
    wb = np.searchsorted(rows, np.arange(0, N_MENTIONS + 1, P)).astype(np.int64)
    counts = np.diff(wb)
    cpw = max(MIN_CPW, int(np.ceil(counts.max() / P)))
    padn = cpw * P

    wid = np.repeat(np.arange(N_WINDOWS, dtype=np.int64), counts)
    pos = np.arange(NNZ, dtype=np.int64) - np.repeat(wb[:-1], counts)

    # pads: code 0 (valid fetch), mention 0, val 0 -> zeroed by sel
    codes_p = np.zeros((N_WINDOWS, padn), np.int32)
    rows_p = np.zeros((N_WINDOWS, padn), np.float32)
    vals_p = np.zeros((N_WINDOWS, padn), np.float32)
    codes_p[wid, pos] = codes.astype(np.int32)
    rows_p[wid, pos] = (rows - wid * P).astype(np.float32)
    vals_p[wid, pos] = vals

    iota = np.ascontiguousarray(
        np.broadcast_to(np.arange(P, dtype=np.float32), (P, P))
    )

    in_maps = []
    for i in range(N_CORES):
        sl = slice(i * WIN_PER_CORE, (i + 1) * WIN_PER_CORE)
        in_maps.append(
            {
                "token_ft": ft,
                "codes": np.ascontiguousarray(
                    codes_p[sl].reshape(-1, P).T
                ),
                "rows": np.ascontiguousarray(rows_p[sl].reshape(-1, P).T),
                "vals": np.ascontiguousarray(vals_p[sl].reshape(-1, P).T),
                "iota": iota,
            }
        )

    if cpw not in _nc_cache:
        _nc_cache[cpw] = _build_nc(cpw)
    nc = _nc_cache[cpw]

    trace = bool(os.environ.get("BASS_KERNEL_TRACE"))
    LAST_RESULTS = run_bass_kernel_spmd(
        nc, in_maps, list(range(N_CORES)), trace=trace
    )
    return np.concatenate(
        [LAST_RESULTS.results[i]["out"] for i in range(N_CORES)], axis=0
    )


# revision 9
# speedup vs baseline: 1.0068x; 1.0068x over previous
"""Trainium2 Bass kernel for nn_Char2Token2Mention (gather + segment-sum).

    ft = token_ft[token_code]               # [NNZ, D] gather
    weighted = ft * spm_vals[:, None]
    out = segment_sum(weighted, spm_rows, num_segments=N_MENTIONS)

Strategy (8-core SPMD, mentions sharded):
  - core i owns mentions [i*8192, (i+1)*8192); spm_rows is sorted so its nnz
    form a contiguous slice.
  - host bins nnz into 128-mention windows (64 per core), pads each window to
    cpw chunks of 128 nnz (pad entries: code 0 with val 0, so they fetch a
    real row that the sel matrix zeroes out -- no bounds check needed).
  - host casts token_ft to bf16 (tolerance is 2e-2; bf16 lands ~2e-3) --
    halves gather DMA traffic and runs the PE at bf16 rate (f32 PSUM).
  - device, per chunk: indirect-DMA gather of 128 rows [128, 256] bf16 from
    the (replicated) token_ft table; DVE builds sel[j, m] = (iota==row_j)
    *val_j; PE matmul sel.T @ ft accumulates the window's [128, 256] f32
    output in PSUM.
  - per window: PSUM -> SBUF copy -> DMA to the core's [8192, 256] output.
  - host concatenates the 8 core outputs.
"""
import os
import numpy as np
import ml_dtypes

import concourse.bacc as bacc
import concourse.bass as bass
import concourse.mybir as mybir
import concourse.tile as tile
from concourse.bass_utils import run_bass_kernel_spmd

P = 128
D = 256
N_TOKENS = 262144
NNZ = 524288
N_MENTIONS = 65536
N_CORES = 8
MENT_PER_CORE = N_MENTIONS // N_CORES          # 8192
WIN_PER_CORE = MENT_PER_CORE // P              # 64
N_WINDOWS = N_MENTIONS // P                    # 512
MIN_CPW = 9

# Results of the last run (set by kernel()); test.py reads exec_time_ns.
LAST_RESULTS = None

_nc_cache = {}


def _build_nc(cpw: int) -> bass.Bass:
    n_chunks = WIN_PER_CORE * cpw
    bf16 = mybir.dt.bfloat16
    nc = bacc.Bacc("TRN2", target_bir_lowering=False, debug=False)
    table = nc.declare_dram_parameter(
        "token_ft", [N_TOKENS, D], bf16, isOutput=False
    )
    codes = nc.declare_dram_parameter(
        "codes", [P, n_chunks], mybir.dt.int32, isOutput=False
    )
    rows = nc.declare_dram_parameter(
        "rows", [P, n_chunks], mybir.dt.float32, isOutput=False
    )
    vals = nc.declare_dram_parameter(
        "vals", [P, n_chunks], mybir.dt.float32, isOutput=False
    )
    iota = nc.declare_dram_parameter(
        "iota", [P, P], mybir.dt.float32, isOutput=False
    )
    out = nc.declare_dram_parameter(
        "out", [MENT_PER_CORE, D], mybir.dt.float32, isOutput=True
    )

    with tile.TileContext(nc) as tc:
        with (
            tc.tile_pool(name="const", bufs=1) as const_pool,
            tc.tile_pool(name="work", bufs=24) as work_pool,
            tc.tile_pool(name="psum", bufs=4, space="PSUM") as psum_pool,
            tc.tile_pool(name="outp", bufs=4) as out_pool,
        ):
            codes_sb = const_pool.tile([P, n_chunks], mybir.dt.int32)
            rows_sb = const_pool.tile([P, n_chunks], mybir.dt.float32)
            vals_sb = const_pool.tile([P, n_chunks], mybir.dt.float32)
            iota_sb = const_pool.tile([P, P], mybir.dt.float32)
            nc.sync.dma_start(out=codes_sb[:], in_=codes[:])
            nc.sync.dma_start(out=rows_sb[:], in_=rows[:])
            nc.sync.dma_start(out=vals_sb[:], in_=vals[:])
            nc.sync.dma_start(out=iota_sb[:], in_=iota[:])

            for w in range(WIN_PER_CORE):
                psum = psum_pool.tile(
                    [P, D], mybir.dt.float32, space="PSUM", tag="acc"
                )
                for c in range(cpw):
                    k = w * cpw + c
                    ft = work_pool.tile([P, D], bf16, tag="ft")
                    nc.gpsimd.indirect_dma_start(
                        out=ft[:],
                        out_offset=None,
                        in_=table[:],
                        in_offset=bass.IndirectOffsetOnAxis(
                            ap=codes_sb[:, k : k + 1], axis=0
                        ),
                    )
                    sel = work_pool.tile([P, P], bf16, tag="sel")
                    nc.vector.tensor_scalar(
                        out=sel[:],
                        in0=iota_sb[:],
                        scalar1=rows_sb[:, k : k + 1],
                        scalar2=vals_sb[:, k : k + 1],
                        op0=mybir.AluOpType.is_equal,
                        op1=mybir.AluOpType.mult,
                    )
                    nc.tensor.matmul(
                        out=psum[:],
                        lhsT=sel[:],
                        rhs=ft[:],
                        start=(c == 0),
                        stop=(c == cpw - 1),
                    )
                outt = out_pool.tile([P, D], mybir.dt.float32, tag="out")
                nc.vector.tensor_copy(out=outt[:], in_=psum[:])
                nc.sync.dma_start(out=out[w * P : (w + 1) * P, :], in_=outt[:])
    nc.compile()
    return nc


def kernel(token_ft, token_code, spm_rows, spm_vals):
    global LAST_RESULTS
    ft = np.ascontiguousarray(
        np.asarray(token_ft, dtype=np.float32).astype(ml_dtypes.bfloat16)
    )
    codes = np.asarray(token_code).astype(np.int64, copy=False)
    rows = np.asarray(spm_rows).astype(np.int64, copy=False)
    vals = np.asarray(spm_vals, dtype=np.float32)
    if not np.all(rows[:-1] <= rows[1:]):
        order = np.argsort(rows, kind="stable")
        rows, codes, vals = rows[order], codes[order], vals[order]

    # bin nnz into 128-mention windows (rows is sorted)
    wb = np.searchsorted(rows, np.arange(0, N_MENTIONS + 1, P)).astype(np.int64)
    counts = np.diff(wb)
    cpw = max(MIN_CPW, int(np.ceil(counts.max() / P)))
    padn = cpw * P

    wid = np.repeat(np.arange(N_WINDOWS, dtype=np.int64), counts)
    pos = np.arange(NNZ, dtype=np.int64) - np.repeat(wb[:-1], counts)

    # pads: code 0 (valid fetch), mention 0, val 0 -> zeroed by sel
    codes_p = np.zeros((N_WINDOWS, padn), np.int32)
    rows_p = np.zeros((N_WINDOWS, padn), np.float32)
    vals_p = np.zeros((N_WINDOWS, padn), np.float32)
    codes_p[wid, pos] = codes.astype(np.int32)
    rows_p[wid, pos] = (rows - wid * P).astype(np.float32)
    vals_p[wid, pos] = vals

    iota = np.ascontiguousarray(
        np.broadcast_to(np.arange(P, dtype=np.float32), (P, P))
    )

    in_maps = []
    for i in range(N_CORES):
        sl = slice(i * WIN_PER_CORE, (i + 1) * WIN_PER_CORE)
        in_maps.append(
            {
                "token_ft": ft,
                "codes": np.ascontiguousarray(
                    codes_p[sl].reshape(-1, P).T
                ),
                "rows": np.ascontiguousarray(rows_p[sl].reshape(-1, P).T),
                "vals": np.ascontiguousarray(vals_p[sl].reshape(-1, P).T),
                "iota": iota,
            }
        )

    if cpw not in _nc_cache:
        _nc_cache[cpw] = _build_nc(cpw)
    nc = _nc_cache[cpw]

    trace = bool(os.environ.get("BASS_KERNEL_TRACE"))
    LAST_RESULTS = run_bass_kernel_spmd(
        nc, in_maps, list(range(N_CORES)), trace=trace
    )
    return np.concatenate(
        [LAST_RESULTS.results[i]["out"] for i in range(N_CORES)], axis=0
    )
